# revision 20
# baseline (speedup 1.0000x reference)
"""BFP activation quantization kernel for 8 TRN2 NeuronCores.

Reference semantics (per (n,h,w) block over C=256 channels):
    max_abs = max_c |x|
    e such that max_abs = m * 2^e, m in [0.5, 1)   (frexp)
    delta = 2^(e-3)
    q = trunc(x / delta) * delta

Sharding: pure data-parallel over N (batch): 64 images -> 8 per core.

Per-core implementation (layout [c -> partitions, hw -> free], fully
contiguous DMA, hw split into half-image chunks). Engine assignment is
driven by measured HW costs (slope microbenchmarks), not the cost
model: partition_all_reduce is superlinear in width (29us at 6272 cols
vs 3.4us at 1568), DVE broadcast multiplies are ~0.4ns/col, and the
scalar engine costs ~0.8ns/col per activation:
  - Pool (GpSimd): ONLY the two per-C-half partition_all_reduce(absmax)
    ops at half-image width (the first overlaps the second half's DMA).
  - DVE: fold of the two half-maxes, exponent mask
    eb = bits(mx) & 0x7f800000, sign extraction sb = (x < 0) as int16,
    t = x * (-recip) (exact: recip is a power of 2), w = sb - r
    (pure int16, fast DVE mode), q = w * delta into the dead x tile.
  - Act (ScalarE): nr = eb * -1 == bits(-recip); db = eb - 0x01000000
    == bits(delta) (exact fp32 integer arithmetic on multiples of
    2^23); and the single convert r = cvt_i16(t + c), c = 0.5 - 2^-25.
Why one convert suffices: t = -u where u = x/delta (exact). For
non-integer u, r = RN(t + c) = ceil(t) = -floor(u), and
trunc(u) = floor(u) + [u<0] = sb - r with sb = signbit(x). Exact
integer u (probability ~2^-21 per element, tens of the 51M elements)
can land one delta step off - negligible vs the 2e-2 rel-err gate.
"""

import sys

for _p in ("/opt/trn_rl_repo", "/root/.axon_site/_ro/trn_rl_repo"):
    if _p not in sys.path:
        sys.path.append(_p)

import numpy as np

N, C, H, W = 64, 256, 56, 56
HW = H * W  # 3136
NCORES = 8
NPC = N // NCORES  # images per core
F = 3136  # hw columns per image
HF = F // 2  # columns per chunk (2 chunks per image)
OB, OC, OD, OE = 1, 2, 3, 4  # stage pipeline offsets (chunks)
BUF_XT, BUF_SR, BUF_M1, BUF_A1 = 6, 3, 6, 5


def _i32(v):
    v &= 0xFFFFFFFF
    return v - (1 << 32) if v >= (1 << 31) else v


_cache = {}


def _build(repeat=1):
    if ("nc", repeat) in _cache:
        return _cache[("nc", repeat)]

    import concourse.bacc as bacc
    import concourse.mybir as mybir
    import concourse.tile as tile
    from concourse import bass_isa

    dt = mybir.dt
    op = mybir.AluOpType

    nc = bacc.Bacc(
        "TRN2",
        target_bir_lowering=False,
        debug=False,
        enable_asserts=False,
        num_devices=NCORES,
    )
    x_d = nc.dram_tensor("x", [NPC, C, HW], dt.float32, kind="ExternalInput").ap()
    y_d = nc.dram_tensor("y", [NPC, C, HW], dt.float32, kind="ExternalOutput").ap()

    NCH = NPC * 2 * repeat

    with tile.TileContext(nc) as tc:
        with (
            tc.tile_pool(name="xtp", bufs=BUF_XT) as xtp,
            tc.tile_pool(name="srp", bufs=BUF_SR) as srp,
            tc.tile_pool(name="m1p", bufs=BUF_M1) as m1p,
            tc.tile_pool(name="a1p", bufs=BUF_A1) as a1p,
            tc.tile_pool(name="consts", bufs=1) as consts,
        ):
            cbias = consts.tile([128, 1], dt.float32)  # +(0.5 - 2^-25)
            nc.gpsimd.memset(cbias[:], 0.4999999701976776123046875)
            dbias = consts.tile([128, 1], dt.float32)  # delta: eb - 0x01000000
            nc.gpsimd.memset(dbias[:], -float(0x01000000))

            # Software-pipelined emission: stage S of chunk k is emitted
            # alongside later stages of older chunks, so every engine's
            # ready queue is ordered by chunk age.
            xts, m1s, a1s, sbs, rrs = {}, {}, {}, {}, {}

            def addr(k):
                n = (k // 2) % NPC
                c0 = (k % 2) * HF
                return n, c0

            for k in range(NCH + OE):
                if OB <= k < NCH + OB:  # stage B: fold, eb, nr, db
                    j = k - OB
                    m1, a1 = m1s[j], a1s[j]
                    nc.vector.tensor_tensor(
                        out=m1[:], in0=m1[:], in1=a1[:], op=op.max
                    )
                    eb = m1[:].bitcast(dt.int32)
                    nc.vector.tensor_scalar(
                        out=eb, in0=eb,
                        scalar1=_i32(0x7F800000), scalar2=None,
                        op0=op.bitwise_and,
                    )
                    # nr = -eb == bits(-recip) into the dead a1 tile;
                    # db = eb - 0x01000000 == bits(delta) in place.
                    nc.scalar.mul(out=a1[:].bitcast(dt.int32), in_=eb, mul=-1.0)
                    nc.scalar.activation(
                        out=eb, in_=eb,
                        func=mybir.ActivationFunctionType.Identity,
                        bias=dbias[:], scale=1.0,
                    )
                if OC <= k < NCH + OC:  # stage C: sign bits, then t
                    j = k - OC
                    xt, a1 = xts[j], a1s[j]
                    sb = sbs[j] = srp.tile(
                        [128, 2 * HF], dt.int16, tag="sb", name=f"sb{j}"
                    )
                    nc.vector.tensor_scalar(
                        out=sb[:], in0=xt[:],
                        scalar1=0.0, scalar2=None, op0=op.is_lt,
                    )
                    nrf = a1[:][:, None, :].broadcast_to([128, 2, HF])
                    x3 = xt[:].rearrange("p (r f) -> p r f", r=2)
                    nc.vector.tensor_tensor(out=x3, in0=x3, in1=nrf, op=op.mult)
                if OD <= k < NCH + OD:  # stage D: the one convert
                    j = k - OD
                    xt = xts[j]
                    rr = rrs[j] = srp.tile(
                        [128, 2 * HF], dt.int16, tag="rr", name=f"rr{j}"
                    )
                    nc.scalar.activation(
                        out=rr[:], in_=xt[:],
                        func=mybir.ActivationFunctionType.Identity,
                        bias=cbias[:], scale=1.0,
                    )
                if k < NCH:  # stage A: loads + per-half partition reduce
                    n, c0 = addr(k)
                    xt = xts[k] = xtp.tile(
                        [128, 2 * HF], dt.float32, tag="xt", name=f"xt{k}"
                    )
                    nc.sync.dma_start(
                        out=xt[:, 0:HF], in_=x_d[n, 0:128, c0 : c0 + HF]
                    )
                    m1 = m1s[k] = m1p.tile(
                        [128, HF], dt.float32, tag="m1", name=f"m1_{k}"
                    )
                    nc.gpsimd.partition_all_reduce(
                        m1[:], xt[:, 0:HF], 128, bass_isa.ReduceOp.absmax
                    )
                    nc.sync.dma_start(
                        out=xt[:, HF : 2 * HF], in_=x_d[n, 128:256, c0 : c0 + HF]
                    )
                    a1 = a1s[k] = a1p.tile(
                        [128, HF], dt.float32, tag="a1", name=f"a1_{k}"
                    )
                    nc.gpsimd.partition_all_reduce(
                        a1[:], xt[:, HF : 2 * HF], 128, bass_isa.ReduceOp.absmax
                    )
                if OE <= k:  # stage E: w, q, stores
                    j = k - OE
                    n, c0 = addr(j)
                    xt, m1 = xts.pop(j), m1s.pop(j)
                    sb, rr = sbs.pop(j), rrs.pop(j)
                    a1s.pop(j, None)
                    # w = sb - r = trunc(x/delta), pure int16, in place.
                    nc.vector.tensor_tensor(
                        out=sb[:], in0=sb[:], in1=rr[:], op=op.subtract
                    )
                    dbf = m1[:][:, None, :].broadcast_to([128, 2, HF])
                    x3 = xt[:].rearrange("p (r f) -> p r f", r=2)
                    nc.vector.tensor_tensor(
                        out=x3, in0=sb[:].rearrange("p (r f) -> p r f", r=2),
                        in1=dbf, op=op.mult,
                    )
                    nc.scalar.dma_start(
                        out=y_d[n, 0:128, c0 : c0 + HF], in_=xt[:, 0:HF]
                    )
                    nc.scalar.dma_start(
                        out=y_d[n, 128:256, c0 : c0 + HF], in_=xt[:, HF : 2 * HF]
                    )
    nc.compile()
    _cache[("nc", repeat)] = nc
    return nc


def _run(x, trace=False, **kwargs):
    from concourse import bass_utils

    nc = _build()
    xs = np.ascontiguousarray(x.reshape(N, C, HW))
    in_maps = [
        {"x": xs[i * NPC : (i + 1) * NPC]} for i in range(NCORES)
    ]
    res = bass_utils.run_bass_kernel_spmd(
        nc, in_maps, core_ids=list(range(NCORES)), trace=trace, **kwargs
    )
    out = np.concatenate([r["y"] for r in res.results], axis=0)
    return out.reshape(N, C, H, W), res


def kernel(activations):
    out, _ = _run(np.asarray(activations))
    return out


# revision 21
# speedup vs baseline: 2.7499x; 2.7499x over previous
"""BFP activation quantization kernel for 8 TRN2 NeuronCores.

Reference semantics (per (n,h,w) block over C=256 channels):
    max_abs = max_c |x|
    e such that max_abs = m * 2^e, m in [0.5, 1)   (frexp)
    delta = 2^(e-3)
    q = trunc(x / delta) * delta

Sharding: pure data-parallel over N (batch): 64 images -> 8 per core.

Per-core implementation (layout [c -> partitions, hw -> free], fully
contiguous DMA, hw split into half-image chunks). Engine assignment is
driven by measured HW costs (slope microbenchmarks), not the cost
model: partition_all_reduce is superlinear in width (29us at 6272 cols
vs 3.4us at 1568), DVE broadcast multiplies are ~0.4ns/col, and the
scalar engine costs ~0.8ns/col per activation:
  - Pool (GpSimd): ONLY the two per-C-half partition_all_reduce(absmax)
    ops at half-image width (the first overlaps the second half's DMA).
  - DVE: fold of the two half-maxes, exponent mask
    eb = bits(mx) & 0x7f800000, sign extraction sb = bits(x) >> 31,
    t = x * (-recip) (exact: recip is a power of 2), w = sb - r
    (pure int16, fast DVE mode), q = w * delta into the dead x tile.
  - Act (ScalarE): nr = eb * -1 == bits(-recip); db = eb - 0x01000000
    == bits(delta) (exact fp32 integer arithmetic on multiples of
    2^23); and the single convert r = cvt_i16(t + c), c = 0.5 - 2^-25.
Why one convert suffices: t = -u where u = x/delta (exact). For
non-integer u, r = RN(t + c) = ceil(t) = -floor(u), and
trunc(u) = floor(u) + [u<0] = sb - r with sb = signbit(x). Exact
integer u (probability ~2^-21 per element, tens of the 51M elements)
can land one delta step off - negligible vs the 2e-2 rel-err gate.
"""

import sys

for _p in ("/opt/trn_rl_repo", "/root/.axon_site/_ro/trn_rl_repo"):
    if _p not in sys.path:
        sys.path.append(_p)

import numpy as np

N, C, H, W = 64, 256, 56, 56
HW = H * W  # 3136
NCORES = 8
NPC = N // NCORES  # images per core
F = 3136  # hw columns per image
HF = F // 2  # columns per chunk (2 chunks per image)
OB, OC, OD, OE = 1, 2, 3, 4  # stage pipeline offsets (chunks)
BUF_XT, BUF_SR, BUF_M1, BUF_A1 = 5, 3, 5, 4


def _i32(v):
    v &= 0xFFFFFFFF
    return v - (1 << 32) if v >= (1 << 31) else v


_cache = {}


def _build(repeat=1):
    if ("nc", repeat) in _cache:
        return _cache[("nc", repeat)]

    import concourse.bacc as bacc
    import concourse.mybir as mybir
    import concourse.tile as tile
    from concourse import bass_isa

    dt = mybir.dt
    op = mybir.AluOpType

    nc = bacc.Bacc(
        "TRN2",
        target_bir_lowering=False,
        debug=False,
        enable_asserts=False,
        num_devices=NCORES,
    )
    x_d = nc.dram_tensor("x", [NPC, C, HW], dt.float32, kind="ExternalInput").ap()
    y_d = nc.dram_tensor("y", [NPC, C, HW], dt.float32, kind="ExternalOutput").ap()

    NCH = NPC * 2 * repeat

    with tile.TileContext(nc) as tc:
        with (
            tc.tile_pool(name="xtp", bufs=BUF_XT) as xtp,
            tc.tile_pool(name="srp", bufs=BUF_SR) as srp,
            tc.tile_pool(name="m1p", bufs=BUF_M1) as m1p,
            tc.tile_pool(name="a1p", bufs=BUF_A1) as a1p,
            tc.tile_pool(name="consts", bufs=1) as consts,
        ):
            cbias = consts.tile([128, 1], dt.float32)  # +(0.5 - 2^-25)
            nc.gpsimd.memset(cbias[:], 0.4999999701976776123046875)
            dbias = consts.tile([128, 1], dt.float32)  # delta: eb - 0x01000000
            nc.gpsimd.memset(dbias[:], -float(0x01000000))

            # Software-pipelined emission: stage S of chunk k is emitted
            # alongside later stages of older chunks, so every engine's
            # ready queue is ordered by chunk age.
            xts, m1s, a1s, sbs, rrs = {}, {}, {}, {}, {}

            def addr(k):
                n = (k // 2) % NPC
                c0 = (k % 2) * HF
                return n, c0

            for k in range(NCH + OE):
                if OB <= k < NCH + OB:  # stage B: fold, eb, nr, db
                    j = k - OB
                    m1, a1 = m1s[j], a1s[j]
                    nc.vector.tensor_tensor(
                        out=m1[:], in0=m1[:], in1=a1[:], op=op.max
                    )
                    eb = m1[:].bitcast(dt.int32)
                    nc.vector.tensor_scalar(
                        out=eb, in0=eb,
                        scalar1=_i32(0x7F800000), scalar2=None,
                        op0=op.bitwise_and,
                    )
                    # nr = -eb == bits(-recip) into the dead a1 tile;
                    # db = eb - 0x01000000 == bits(delta) in place.
                    nc.scalar.mul(out=a1[:].bitcast(dt.int32), in_=eb, mul=-1.0)
                    nc.scalar.activation(
                        out=eb, in_=eb,
                        func=mybir.ActivationFunctionType.Identity,
                        bias=dbias[:], scale=1.0,
                    )
                if OC <= k < NCH + OC:  # stage C: sign bits, then t
                    j = k - OC
                    xt, a1 = xts[j], a1s[j]
                    sb = sbs[j] = srp.tile(
                        [128, 2 * HF], dt.int32, tag="sb", name=f"sb{j}"
                    )
                    nc.vector.tensor_scalar(
                        out=sb[:], in0=xt[:].bitcast(dt.int32),
                        scalar1=31, scalar2=None, op0=op.logical_shift_right,
                    )
                    nrf = a1[:][:, None, :].broadcast_to([128, 2, HF])
                    x3 = xt[:].rearrange("p (r f) -> p r f", r=2)
                    nc.vector.tensor_tensor(out=x3, in0=x3, in1=nrf, op=op.mult)
                if OD <= k < NCH + OD:  # stage D: the one convert
                    j = k - OD
                    xt = xts[j]
                    rr = rrs[j] = srp.tile(
                        [128, 2 * HF], dt.int16, tag="rr", name=f"rr{j}"
                    )
                    nc.scalar.activation(
                        out=rr[:], in_=xt[:],
                        func=mybir.ActivationFunctionType.Identity,
                        bias=cbias[:], scale=1.0,
                    )
                if k < NCH:  # stage A: loads + per-half partition reduce
                    n, c0 = addr(k)
                    xt = xts[k] = xtp.tile(
                        [128, 2 * HF], dt.float32, tag="xt", name=f"xt{k}"
                    )
                    nc.sync.dma_start(
                        out=xt[:, 0:HF], in_=x_d[n, 0:128, c0 : c0 + HF]
                    )
                    m1 = m1s[k] = m1p.tile(
                        [128, HF], dt.float32, tag="m1", name=f"m1_{k}"
                    )
                    nc.gpsimd.partition_all_reduce(
                        m1[:], xt[:, 0:HF], 128, bass_isa.ReduceOp.absmax
                    )
                    nc.sync.dma_start(
                        out=xt[:, HF : 2 * HF], in_=x_d[n, 128:256, c0 : c0 + HF]
                    )
                    a1 = a1s[k] = a1p.tile(
                        [128, HF], dt.float32, tag="a1", name=f"a1_{k}"
                    )
                    nc.gpsimd.partition_all_reduce(
                        a1[:], xt[:, HF : 2 * HF], 128, bass_isa.ReduceOp.absmax
                    )
                if OE <= k:  # stage E: w, q, stores
                    j = k - OE
                    n, c0 = addr(j)
                    xt, m1 = xts.pop(j), m1s.pop(j)
                    sb, rr = sbs.pop(j), rrs.pop(j)
                    a1s.pop(j, None)
                    # w = sb - r = trunc(x/delta), pure int16, in place.
                    nc.vector.tensor_tensor(
                        out=sb[:], in0=sb[:], in1=rr[:], op=op.subtract
                    )
                    dbf = m1[:][:, None, :].broadcast_to([128, 2, HF])
                    x3 = xt[:].rearrange("p (r f) -> p r f", r=2)
                    nc.vector.tensor_tensor(
                        out=x3, in0=sb[:].rearrange("p (r f) -> p r f", r=2),
                        in1=dbf, op=op.mult,
                    )
                    nc.scalar.dma_start(
                        out=y_d[n, 0:128, c0 : c0 + HF], in_=xt[:, 0:HF]
                    )
                    nc.scalar.dma_start(
                        out=y_d[n, 128:256, c0 : c0 + HF], in_=xt[:, HF : 2 * HF]
                    )
    nc.compile()
    _cache[("nc", repeat)] = nc
    return nc


def _run(x, trace=False, **kwargs):
    from concourse import bass_utils

    nc = _build()
    xs = np.ascontiguousarray(x.reshape(N, C, HW))
    in_maps = [
        {"x": xs[i * NPC : (i + 1) * NPC]} for i in range(NCORES)
    ]
    res = bass_utils.run_bass_kernel_spmd(
        nc, in_maps, core_ids=list(range(NCORES)), trace=trace, **kwargs
    )
    out = np.concatenate([r["y"] for r in res.results], axis=0)
    return out.reshape(N, C, H, W), res


def kernel(activations):
    out, _ = _run(np.asarray(activations))
    return out
